# revision 2
# baseline (speedup 1.0000x reference)
"""Trainium2 Bass kernel for nn_ByteFormerWrapper (block_size=4096).

Math: reference computes img = byte2image_4k(x) (B,8,128,496) then
out = einsum('bchw,wo->bcho', img, W).

Key identity: img[b, c, p*8+s, i] = shifts_s[b, c, i+p] where
shifts_s[b, c, j] = ((F >> (8-s)) & 255), F = 256*x[b,512c+j] + x[b,512c+j+1]
(next byte zero at j=511, per 512-byte sub-block), for i in [0,496),
p in [0,16), s in [0,8). Since i+p <= 510 there is no wraparound.

So with norm(v) = v*(2/255) - 1:
  out[b,c,p*8+s,o] = (2/255) * sum_j shifts_s[b,c,j] * Wpad_p[j,o] - S[o]
where Wpad_p is W zero-padded to 512 rows with offset p, S = W.sum(0).

Device program (per core, 32 batch rows => 256 (b,c) sub-blocks):
  1. Load x as int32 [256,512] (partition=(b,c) in 2 chunks of 128).
  2. DVE: F = (U << 8) | V;  sht_s = f16((F >> (8-s)) & 255)  [128,512].
  3. PE transpose 128x128 blocks -> AT[k][j_local, s*256+bc] f16, k=0..3.
  4. Matmul: stationary = [Wpad_{2q} | Wpad_{2q+1}] chunk [128j,128m] f16,
     moving = AT[k][:, 512n:512n+512]; accumulate k=0..3 into PSUM [128,512].
  5. DVE eviction: out = psum*(2/255) - S2[partition], f32.
  6. DMA to OT [16,64,2048] = [p, o, (s,bc)].
Host reassembles OT -> (32,8,128,64) per core, concat over 8 cores.
"""

import numpy as np

NCORES = 8
B = 256
B_LOC = B // NCORES  # 32 batch rows per core
NGRAM = 16
SUB = 512

_CACHE = {}


def _build_program(repeat=1):
    import concourse.mybir as mybir
    import concourse.tile as tile
    from concourse import bacc
    from concourse.masks import make_identity

    f32 = mybir.dt.float32
    f16 = mybir.dt.float16
    i32 = mybir.dt.int32

    nc = bacc.Bacc(None, target_bir_lowering=False, debug=False)

    with tile.TileContext(nc) as tc:
        with tc.tile_pool(name="dram", bufs=1, space="DRAM") as dram:
            x_d = dram.tile([256, 512], i32, kind="ExternalInput", name="x", uniquify=False)
            ws_d = dram.tile([128, 8, 4, 128], f16, kind="ExternalInput", name="ws", uniquify=False)
            bias_d = dram.tile([128, 1], f32, kind="ExternalInput", name="bias", uniquify=False)
            ot_d = dram.tile([16, 64, 2048], f32, kind="ExternalOutput", name="ot", uniquify=False)
            ot_flat = ot_d.rearrange("p o n -> (p o) n")

            with (
                tc.tile_pool(name="const", bufs=1) as constp,
                tc.tile_pool(name="xin", bufs=2) as xinp,
                tc.tile_pool(name="sh", bufs=3) as shp,
                tc.tile_pool(name="at", bufs=1) as atp,
                tc.tile_pool(name="tpsum", bufs=2, space="PSUM") as tpsum,
                tc.tile_pool(name="mpsum", bufs=4, space="PSUM") as mpsum,
                tc.tile_pool(name="oev", bufs=4) as oevp,
            ):
                ident = constp.tile([128, 128], f16)
                make_identity(nc, ident)
                ws_sb = constp.tile([128, 8, 4, 128], f16)
                nc.sync.dma_start(ws_sb[:], ws_d[:])
                bias_sb = constp.tile([128, 1], f32)
                nc.sync.dma_start(bias_sb[:], bias_d[:])

                def body():
                    at = [atp.tile([128, 2048], f16, name=f"at{k}") for k in range(4)]
                    for h in range(2):  # (b,c) chunk of 128
                        U = xinp.tile([128, 512], i32, name="U")
                        nc.sync.dma_start(U[:], x_d[128 * h:128 * (h + 1), :])
                        V = xinp.tile([128, 512], i32, name="V")
                        nc.sync.dma_start(V[:, 0:511], x_d[128 * h:128 * (h + 1), 1:512])
                        nc.vector.memset(V[:, 511:512], 0)
                        T8 = xinp.tile([128, 512], i32, name="T8")
                        nc.vector.tensor_scalar(
                            T8[:], U[:], 8, None, op0=mybir.AluOpType.logical_shift_left
                        )
                        F = xinp.tile([128, 512], i32, name="F")
                        nc.vector.tensor_tensor(F[:], T8[:], V[:], op=mybir.AluOpType.bitwise_or)
                        for s in range(8):
                            sht_i = shp.tile([128, 512], i32, name="sht_i")
                            nc.vector.tensor_scalar(
                                sht_i[:], F[:], 8 - s, 255,
                                op0=mybir.AluOpType.logical_shift_right,
                                op1=mybir.AluOpType.bitwise_and,
                            )
                            sht = shp.tile([128, 512], f16, name="sht")
                            nc.vector.tensor_copy(sht[:], sht_i[:])
                            for k in range(4):
                                pt = tpsum.tile([128, 128], f16, name="pt")
                                nc.tensor.transpose(pt[:], sht[:, 128 * k:128 * (k + 1)], ident[:])
                                nc.scalar.copy(at[k][:, s * 256 + 128 * h: s * 256 + 128 * h + 128], pt[:])

                    for q in range(8):  # p-pair (2q, 2q+1)
                        for n in range(4):  # N chunk of 512 columns
                            ps = mpsum.tile([128, 512], f32, name="ps")
                            for k in range(4):  # K chunk of 128 j-rows
                                nc.tensor.matmul(
                                    ps[:],
                                    ws_sb[:, q, k, :],
                                    at[k][:, 512 * n:512 * (n + 1)],
                                    start=(k == 0),
                                    stop=(k == 3),
                                )
                            ev = oevp.tile([128, 512], f32, name="ev")
                            nc.vector.tensor_scalar(
                                ev[:], ps[:], 2.0 / 255.0, bias_sb[:],
                                op0=mybir.AluOpType.mult,
                                op1=mybir.AluOpType.subtract,
                            )
                            nc.sync.dma_start(
                                ot_flat[128 * q:128 * (q + 1), 512 * n:512 * (n + 1)], ev[:]
                            )

                if repeat == 1:
                    body()
                else:
                    with tc.For_i(0, repeat):
                        body()

    nc.finalize()
    return nc


def _prep_inputs(x, W):
    """Host-side prep: per-core int32 x views + replicated f16 weight tensors."""
    x_i32 = np.ascontiguousarray(x.astype(np.int32).reshape(B, 8, SUB))
    W = np.asarray(W, dtype=np.float32)

    # ws[j_local, q, k, m]: m = 64*t + o -> Wpad_{2q+t}[128*k + j_local, o]
    wpad = np.zeros((16, 512, 64), np.float32)
    for p in range(16):
        wpad[p, p:p + 496, :] = W
    ws = np.zeros((128, 8, 4, 128), np.float32)
    for q in range(8):
        for k in range(4):
            for t in range(2):
                ws[:, q, k, 64 * t:64 * t + 64] = wpad[2 * q + t, 128 * k:128 * (k + 1), :]
    ws = ws.astype(np.float16)

    s_col = ws.astype(np.float32).reshape(128 * 8 * 4, 128).sum()  # unused; bias from W directly
    bias = np.tile(np.asarray(W, np.float32).sum(0), 2).reshape(128, 1).astype(np.float32)

    in_maps = []
    for r in range(NCORES):
        xl = np.ascontiguousarray(
            x_i32[r * B_LOC:(r + 1) * B_LOC].reshape(B_LOC * 8, SUB)
        )
        in_maps.append({"x": xl, "ws": ws, "bias": bias})
    return in_maps


def _assemble(results):
    """Per-core OT [16,64,2048] -> (256,8,128,64) f32."""
    outs = []
    for r in range(NCORES):
        ot = np.asarray(results[r]["ot"], dtype=np.float32)
        o5 = ot.reshape(16, 64, 8, B_LOC, 8)          # [p, o, s, b_loc, c]
        outs.append(np.ascontiguousarray(o5.transpose(3, 4, 0, 2, 1)).reshape(B_LOC, 8, 128, 64))
    return np.concatenate(outs, axis=0)


def kernel(x, W):
    from concourse.bass_utils import run_bass_kernel_spmd

    if "nc" not in _CACHE:
        _CACHE["nc"] = _build_program(repeat=1)
    nc = _CACHE["nc"]
    in_maps = _prep_inputs(np.asarray(x), np.asarray(W))
    res = run_bass_kernel_spmd(nc, in_maps, core_ids=list(range(NCORES)))
    return _assemble(res.results)
